# revision 1
# baseline (speedup 1.0000x reference)
"""Trainium2 Bass kernel for nn_BloqueAttn: causal RoPE attention, 16 heads,
head-sharded (tensor-parallel) across 8 NeuronCores, o_proj row-sharded with
host-side all-reduce of the partials.

Self-contained: hardcodes shapes B=1, L=4096, D=1024, H=16, DH=64, 8 cores.
"""
import os

os.environ.setdefault("BASS_NEVER_TRACE", "1")

import numpy as np
import ml_dtypes

import concourse.bass as bass
import concourse.bacc as bacc
import concourse.mybir as mybir
import concourse.tile as tile
from concourse.bass_utils import run_bass_kernel_spmd

F32 = mybir.dt.float32
F32R = mybir.dt.float32r
BF16 = mybir.dt.bfloat16

B, L, D = 1, 4096, 1024
H, DH = 16, 64
BASE = 10000.0
N_CORES = 8
HPC = H // N_CORES          # heads per core = 2
DH2 = HPC * DH              # packed head dim = 128
SCALE = DH ** -0.5          # 0.125

NEG = -1e30


# ---------------------------------------------------------------- host helpers

def _rope_tables(L_, dh):
    inv_freq = 1.0 / (BASE ** (np.arange(0, dh, 2, dtype=np.float32) / dh))
    freqs = np.outer(np.arange(L_, dtype=np.float32), inv_freq)  # [L, 32]
    return np.cos(freqs).astype(np.float32), np.sin(freqs).astype(np.float32)


def _host_consts(L_):
    cos, sin = _rope_tables(L_, DH)          # [L, 32]
    cosT, sinT = cos.T.copy(), sin.T.copy()  # [32, L]
    cos_stack = np.concatenate([cosT, cosT, cosT, cosT], 0)          # [128, L]
    sin_signed = np.concatenate([-sinT, sinT, -sinT, sinT], 0)       # [128, L]

    j = np.arange(128)[:, None]
    c = np.arange(128)[None, :]
    trilneg = np.where(j > c, NEG, 0.0).astype(np.float32)           # [128,128]
    r3mask = np.concatenate(
        [np.full((128, 128), NEG, np.float32), trilneg], axis=1)     # [128,256]

    ident = np.eye(128, dtype=np.float32)
    sel2 = np.zeros((2, 128), np.float32)
    sel2[0, 0:64] = 1.0
    sel2[1, 64:128] = 1.0
    return {
        "cos_st": cos_stack,
        "sin_st": sin_signed,
        "trilneg": trilneg.astype(ml_dtypes.bfloat16),
        "r3mask": r3mask.astype(ml_dtypes.bfloat16),
        "ident_b": ident.astype(ml_dtypes.bfloat16),
        "ident_f": ident,
        "sel2": sel2,
    }


def _core_weights(core, Wq, Wk, Wv, Wo):
    """Per-core transposed weight slices with RoPE even/odd permutation."""
    perm = np.concatenate([np.arange(0, DH, 2), np.arange(1, DH, 2)])  # [64]
    rows_p, rows = [], []
    for hh in (HPC * core, HPC * core + 1):
        rows_p.append(DH * hh + perm)
        rows.append(DH * hh + np.arange(DH))
    rows_p = np.concatenate(rows_p)
    rows = np.concatenate(rows)
    wqT = np.ascontiguousarray(Wq[rows_p, :].T)   # [D, 128]
    wkT = np.ascontiguousarray(Wk[rows_p, :].T)   # [D, 128]
    wvT = np.ascontiguousarray(Wv[rows, :].T)     # [D, 128]
    woC = np.ascontiguousarray(Wo[:, DH2 * core: DH2 * (core + 1)].T)  # [128, D]
    return wqT, wkT, wvT, woC


# ---------------------------------------------------------------- device emit

def emit(nc, tc, aps, L_):
    """Emit the per-core program. aps: dict of dram APs."""
    NLT = L_ // 512           # 512-wide L tiles
    ND = D // 128             # D chunks = 8
    NQB = NLT                 # q tiles of 512

    xt = aps["xt"]
    partial = aps["partial"]

    with tc.tile_pool(name="persist", bufs=1) as pp:
        wq_c = [pp.tile([128, 128], F32R, name=f"wq_c{i}") for i in range(ND)]
        wk_c = [pp.tile([128, 128], F32R, name=f"wk_c{i}") for i in range(ND)]
        wv_c = [pp.tile([128, 128], F32R, name=f"wv_c{i}") for i in range(ND)]
        wo_sb = pp.tile([128, D], F32R)
        cos_sb = pp.tile([128, L_], F32)
        sin_sb = pp.tile([128, L_], F32)
        tril_sb = pp.tile([128, 128], BF16)
        r3m_sb = pp.tile([128, 256], BF16)
        idb_sb = pp.tile([128, 128], BF16)
        idf_sb = pp.tile([128, 128], F32)
        sel2_sb = pp.tile([2, 128], F32R)
        qT = pp.tile([128, L_], F32R)
        kT = pp.tile([128, L_], F32R)
        v_nat = pp.tile([128, (L_ // 128) * 130], F32R)
        OT = pp.tile([128, L_], F32R)
        stack16 = pp.tile([128, 512], F32)
        inv16 = pp.tile([128, 512], F32R)
        inv2 = pp.tile([2, L_], F32R)

        for name, dsts in (("wq", wq_c), ("wk", wk_c), ("wv", wv_c)):
            for ch in range(ND):
                nc.sync.dma_start(dsts[ch][:], aps[name][bass.ts(ch, 128), :])
        nc.sync.dma_start(cos_sb[:], aps["cos_st"][:])
        nc.sync.dma_start(sin_sb[:], aps["sin_st"][:])
        nc.sync.dma_start(wo_sb[:], aps["wo"][:])
        nc.sync.dma_start(tril_sb[:], aps["trilneg"][:])
        nc.sync.dma_start(r3m_sb[:], aps["r3mask"][:])
        nc.sync.dma_start(idb_sb[:], aps["ident_b"][:])
        nc.sync.dma_start(idf_sb[:], aps["ident_f"][:])
        nc.sync.dma_start(sel2_sb[:], aps["sel2"][:])
        nc.gpsimd.memset(v_nat[:].bitcast(F32), 1.0)

        # ---------------- Phase A: projections + RoPE + V transpose ----------
        with tc.tile_pool(name="psA", bufs=1, space="PSUM") as psA, \
             tc.tile_pool(name="sbA", bufs=1) as sbA:
            for lp in range(NLT // 2):
                sl = bass.ts(lp, 1024)
                pr = [psA.tile([128, 1024], F32, tag=t, bufs=1, name=f"pr_{t}")
                      for t in ("qps", "kps", "vps")]
                for ch in range(ND):
                    xt_t = sbA.tile([128, 1024], F32R, tag="xt", bufs=8)
                    nc.sync.dma_start(xt_t[:], xt[bass.ts(ch, 128), sl])
                    st, sp = ch == 0, ch == ND - 1
                    for wgt, ps in zip((wq_c, wk_c, wv_c), pr):
                        for half in range(2):
                            nc.tensor.matmul(ps[:, bass.ts(half, 512)], wgt[ch][:],
                                             xt_t[:, bass.ts(half, 512)],
                                             start=st, stop=sp)
                # RoPE per lp: rot = raw*cos + swap(raw)*sin_signed
                for pi, dst in ((0, qT), (1, kT)):
                    raw = sbA.tile([128, 1024], F32, tag="raw", bufs=2)
                    swp = sbA.tile([128, 1024], F32, tag="swp", bufs=2)
                    nc.scalar.copy(raw[:], pr[pi][:])
                    for b0 in range(4):
                        src_b = (b0 ^ 1) * 32
                        nc.sync.dma_start(swp[b0 * 32:(b0 + 1) * 32, :],
                                          raw[src_b:src_b + 32, :])
                    nc.vector.tensor_mul(dst[:, sl], raw[:], cos_sb[:, sl])
                    nc.vector.tensor_mul(swp[:], swp[:], sin_sb[:, sl])
                    nc.vector.tensor_add(dst[:, sl], dst[:, sl], swp[:])
                # V via PE transpose
                vt = sbA.tile([128, 1024], F32, tag="vt", bufs=2)
                nc.scalar.copy(vt[:], pr[2][:])
                for j in range(8):
                    kb = 8 * lp + j
                    trp = psA.tile([128, 128], F32, tag="trp", bufs=2)
                    nc.tensor.transpose(trp[:], vt[:, bass.ts(j, 128)], idf_sb[:])
                    nc.scalar.copy(v_nat[:, 130 * kb:130 * kb + 64], trp[:, 0:64])
                    nc.scalar.copy(v_nat[:, 130 * kb + 65:130 * kb + 129], trp[:, 64:128])

        # ---------------- Phase B: attention + pipelined norm/o_proj ---------
        def norm_and_oproj(psNC, sbC, qb_lo, qb_hi, base=None):
            if base is None:
                base = 32 * (qb_lo // (NQB // 4)) if NQB >= 4 else 0
            nh = qb_hi - qb_lo
            for qb in range(qb_lo, qb_hi):
                r0 = base + (qb - qb_lo)
                r1 = base + nh + (qb - qb_lo)
                nc.sync.dma_start(stack16[r0:r0 + 1, :], sums_sb[qb][64:65, 0:512])
                nc.sync.dma_start(stack16[r1:r1 + 1, :], sums_sb[qb][64:65, 512:1024])
            with nc.allow_low_precision(reason="f32r reciprocal for bcast matmul"):
                nc.vector.reciprocal(inv16[base:base + 2 * nh, :],
                                     stack16[base:base + 2 * nh, :])
            for qb in range(qb_lo, qb_hi):
                osl = bass.ts(qb, 512)
                r0 = base + (qb - qb_lo)
                r1 = base + nh + (qb - qb_lo)
                nc.sync.dma_start(inv2[0:1, osl], inv16[r0:r0 + 1, :])
                nc.sync.dma_start(inv2[1:2, osl], inv16[r1:r1 + 1, :])
            for qb in range(qb_lo, qb_hi):
                osl = bass.ts(qb, 512)
                bc = psNC.tile([128, 512], F32, tag="op", bufs=2)
                nc.tensor.matmul(bc[:], sel2_sb[:], inv2[:, osl],
                                 start=True, stop=True)
                nc.vector.tensor_mul(OT[:, osl], OT[:, osl], bc[:])
            for lc in range(4 * qb_lo, 4 * qb_hi):
                ob = sbC.tile([128, 1024], F32, tag="ob")
                for n in range(D // 512):
                    op = psNC.tile([128, 512], F32, tag="op", bufs=2)
                    nc.tensor.matmul(op[:], OT[:, bass.ts(lc, 128)],
                                     wo_sb[:, bass.ts(n, 512)],
                                     start=True, stop=True)
                    nc.vector.tensor_copy(ob[:, bass.ts(n, 512)], op[:])
                nc.sync.dma_start(partial[bass.ts(lc, 128), :], ob[:])

        sums_sb = {}
        with tc.tile_pool(name="psB", bufs=1, space="PSUM") as psB, \
             tc.tile_pool(name="psNC", bufs=1, space="PSUM") as psNC, \
             tc.tile_pool(name="sbC", bufs=3) as sbC, \
             tc.tile_pool(name="sbB", bufs=4) as sbB:
            for qb in range(NQB):
                qsl0 = 512 * qb
                ov0 = psB.tile([128, 512], F32, tag="pv0", bufs=1)
                ov1 = psB.tile([128, 512], F32, tag="pv1", bufs=1)
                nkb = 4 * qb + 4
                for kb in range(nkb):
                    r = kb - 4 * qb
                    if r < 0:
                        c0 = 0
                    elif r < 3:
                        c0 = 128 * r
                    else:
                        c0 = 256
                    W = 512 - c0
                    ksl = bass.ts(kb, 128)
                    qsl = bass.ds(qsl0 + c0, W)
                    s01 = psB.tile([128, 1024], F32, tag="sc", bufs=2)
                    diag = r >= 0
                    nc.tensor.matmul(s01[:, c0:512], kT[0:64, ksl],
                                     qT[0:64, qsl], start=True, stop=not diag)
                    nc.tensor.matmul(s01[:, 512 + c0:1024], kT[64:128, ksl],
                                     qT[64:128, qsl], start=True, stop=not diag)
                    if diag:
                        if r < 3:
                            msl, mk = slice(128 * r, 128 * r + 128), tril_sb
                        else:
                            msl, mk = slice(256, 512), r3m_sb
                        nc.tensor.matmul(s01[:, msl], idb_sb[:], mk[:],
                                         start=False, stop=True)
                        nc.tensor.matmul(s01[:, msl.start + 512:msl.stop + 512],
                                         idb_sb[:], mk[:], start=False, stop=True)
                    p01 = sbB.tile([128, 1024], F32R, tag="p01", bufs=8)
                    sin_ = s01[:].rearrange("p (h c) -> p h c", h=2)[:, :, c0:512]
                    pout = p01[:].rearrange("p (h c) -> p h c", h=2)[:, :, c0:512]
                    nc.scalar.activation(pout, sin_,
                                         mybir.ActivationFunctionType.Exp, scale=SCALE)
                    st, sp = kb == 0, kb == nkb - 1
                    nc.tensor.matmul(ov0[0:65, c0:512],
                                     v_nat[:, bass.ds(130 * kb, 65)],
                                     p01[:, c0:512], start=st, stop=sp)
                    nc.tensor.matmul(ov1[0:65, c0:512],
                                     v_nat[:, bass.ds(130 * kb + 65, 65)],
                                     p01[:, 512 + c0:1024], start=st, stop=sp)
                osl = bass.ds(qsl0, 512)
                nc.vector.tensor_copy(OT[0:64, osl], ov0[0:64, :])
                o1t = sbB.tile([64, 512], F32R, tag="o1t", bufs=2)
                nc.vector.tensor_copy(o1t[:], ov1[0:64, :])
                nc.sync.dma_start(OT[64:128, osl], o1t[:])
                sm = sbB.tile([65, 1024], F32, tag="sums", bufs=2)
                sums_sb[qb] = sm
                nc.vector.tensor_copy(sm[64:65, 0:512], ov0[64:65, :])
                nc.vector.tensor_copy(sm[64:65, 512:1024], ov1[64:65, :])
                if NQB >= 4:
                    step = NQB // 4
                    if qb >= NQB - step:
                        # last quarter: per-qb for a shorter serial tail
                        norm_and_oproj(psNC, sbC, qb, qb + 1,
                                       base=32 * (qb - (NQB - step)))
                    elif (qb + 1) % step == 0:
                        norm_and_oproj(psNC, sbC, qb + 1 - step, qb + 1)
                else:
                    if qb == NQB - 1:
                        norm_and_oproj(psNC, sbC, 0, NQB)


def build(L_=L, debug=False):
    nc = bacc.Bacc("TRN2", target_bir_lowering=False, debug=debug,
                   enable_asserts=False)
    aps = {}
    aps["xt"] = nc.dram_tensor("xt", [D, L_], F32R, kind="ExternalInput").ap()
    aps["wq"] = nc.dram_tensor("wq", [D, 128], F32R, kind="ExternalInput").ap()
    aps["wk"] = nc.dram_tensor("wk", [D, 128], F32R, kind="ExternalInput").ap()
    aps["wv"] = nc.dram_tensor("wv", [D, 128], F32R, kind="ExternalInput").ap()
    aps["wo"] = nc.dram_tensor("wo", [128, D], F32R, kind="ExternalInput").ap()
    aps["cos_st"] = nc.dram_tensor("cos_st", [128, L_], F32, kind="ExternalInput").ap()
    aps["sin_st"] = nc.dram_tensor("sin_st", [128, L_], F32, kind="ExternalInput").ap()
    aps["trilneg"] = nc.dram_tensor("trilneg", [128, 128], BF16, kind="ExternalInput").ap()
    aps["r3mask"] = nc.dram_tensor("r3mask", [128, 256], BF16, kind="ExternalInput").ap()
    aps["ident_b"] = nc.dram_tensor("ident_b", [128, 128], BF16, kind="ExternalInput").ap()
    aps["ident_f"] = nc.dram_tensor("ident_f", [128, 128], F32, kind="ExternalInput").ap()
    aps["sel2"] = nc.dram_tensor("sel2", [2, 128], F32R, kind="ExternalInput").ap()
    aps["partial"] = nc.dram_tensor("partial", [L_, D], F32, kind="ExternalOutput").ap()

    with tile.TileContext(nc) as tc:
        emit(nc, tc, aps, L_)
    nc.compile()
    return nc, aps


def make_in_maps(x, Wq, Wk, Wv, Wo, L_=L):
    xT = np.ascontiguousarray(x.reshape(L_, D).T).astype(np.float32)
    consts = _host_consts(L_)
    in_maps = []
    for c in range(N_CORES):
        wqT, wkT, wvT, woC = _core_weights(c, Wq, Wk, Wv, Wo)
        m = {"xt": xT, "wq": wqT, "wk": wkT, "wv": wvT, "wo": woC}
        m.update(consts)
        in_maps.append(m)
    return in_maps


_CACHE = {}


def _run(inputs, trace=False, **kw):
    if trace:
        os.environ.pop("BASS_NEVER_TRACE", None)
    x = np.asarray(inputs["x"], np.float32)
    Wq = np.asarray(inputs["Wq"], np.float32)
    Wk = np.asarray(inputs["Wk"], np.float32)
    Wv = np.asarray(inputs["Wv"], np.float32)
    Wo = np.asarray(inputs["Wo"], np.float32)
    if "nc" not in _CACHE:
        _CACHE["nc"] = build()[0]
    nc = _CACHE["nc"]
    in_maps = make_in_maps(x, Wq, Wk, Wv, Wo)
    res = run_bass_kernel_spmd(nc, in_maps, core_ids=list(range(N_CORES)),
                               trace=trace, **kw)
    acc = np.zeros((L, D), np.float64)
    for r in res.results:
        acc += r["partial"].astype(np.float64)
    out = acc.astype(np.float32).reshape(B, L, D)
    return out, res


def kernel(**inputs):
    out, _ = _run(inputs)
    return out



# revision 2
# speedup vs baseline: 1.2998x; 1.2998x over previous
"""Trainium2 Bass kernel for nn_BloqueAttn: causal RoPE attention, 16 heads,
head-sharded (tensor-parallel) across 8 NeuronCores, o_proj row-sharded with
host-side all-reduce of the partials.

v2: bf16 datapath, query-on-partition PV (65-wide moving operand), PE
perm-matmul RoPE swap, mask-by-multiply on DVE, per-partition softmax
normalization, batched DMAs with host-side pre-layout.

Self-contained: hardcodes shapes B=1, L=4096, D=1024, H=16, DH=64, 8 cores.
"""
import os

os.environ.setdefault("BASS_NEVER_TRACE", "1")

import numpy as np
import ml_dtypes

import concourse.bass as bass
import concourse.bacc as bacc
import concourse.mybir as mybir
import concourse.tile as tile
from concourse.bass_utils import run_bass_kernel_spmd

F32 = mybir.dt.float32
BF16 = mybir.dt.bfloat16

B, L, D = 1, 4096, 1024
H, DH = 16, 64
BASE = 10000.0
N_CORES = 8
HPC = H // N_CORES          # heads per core = 2
DH2 = HPC * DH              # packed head dim = 128
SCALE = DH ** -0.5          # 0.125


# ---------------------------------------------------------------- host helpers

def _rope_tables(L_, dh):
    inv_freq = 1.0 / (BASE ** (np.arange(0, dh, 2, dtype=np.float32) / dh))
    freqs = np.outer(np.arange(L_, dtype=np.float32), inv_freq)  # [L, 32]
    return np.cos(freqs).astype(np.float32), np.sin(freqs).astype(np.float32)


def _host_consts(L_):
    cos, sin = _rope_tables(L_, DH)          # [L, 32]
    cosT, sinT = cos.T.copy(), sin.T.copy()  # [32, L]
    cos_stack = np.concatenate([cosT, cosT, cosT, cosT], 0)          # [128, L]
    sin_signed = np.concatenate([-sinT, sinT, -sinT, sinT], 0)       # [128, L]

    # 0/1 causal keep-mask within a 128x128 diagonal block:
    # key j visible to query c iff j <= c.
    j = np.arange(128)[:, None]
    c = np.arange(128)[None, :]
    tril01 = (j <= c).astype(np.float32)                             # [128,128]

    ident = np.eye(128, dtype=np.float32)
    # 32-row block swap permutation: out[i] = in[sigma(i)],
    # sigma = [32..63, 0..31, 96..127, 64..95]
    sigma = np.concatenate([np.arange(32, 64), np.arange(0, 32),
                            np.arange(96, 128), np.arange(64, 96)])
    pmat = np.zeros((128, 128), np.float32)
    pmat[sigma, np.arange(128)] = 1.0        # out = pmat.T @ in
    return {
        "cos_st": cos_stack.astype(ml_dtypes.bfloat16),
        "sin_st": sin_signed.astype(ml_dtypes.bfloat16),
        "tril01": tril01.astype(ml_dtypes.bfloat16),
        "ident_b": ident.astype(ml_dtypes.bfloat16),
        "perm_b": pmat.astype(ml_dtypes.bfloat16),
    }


def _chunk_major(wT):
    """[D, 128] -> [128, D] with 128-row chunks laid side by side."""
    ndc = wT.shape[0] // 128
    return np.ascontiguousarray(
        wT.reshape(ndc, 128, 128).transpose(1, 0, 2).reshape(128, ndc * 128))


def _core_weights(core, Wq, Wk, Wv, Wo):
    """Per-core weight slices, bf16, chunk-major; RoPE even/odd permutation
    applied to Wq/Wk rows."""
    perm = np.concatenate([np.arange(0, DH, 2), np.arange(1, DH, 2)])  # [64]
    rows_p, rows = [], []
    for hh in (HPC * core, HPC * core + 1):
        rows_p.append(DH * hh + perm)
        rows.append(DH * hh + np.arange(DH))
    rows_p = np.concatenate(rows_p)
    rows = np.concatenate(rows)
    wq = _chunk_major(Wq[rows_p, :].T).astype(ml_dtypes.bfloat16)  # [128, 1024]
    wk = _chunk_major(Wk[rows_p, :].T).astype(ml_dtypes.bfloat16)
    wv = _chunk_major(Wv[rows, :].T).astype(ml_dtypes.bfloat16)
    woC = np.ascontiguousarray(
        Wo[:, DH2 * core: DH2 * (core + 1)].T).astype(ml_dtypes.bfloat16)
    return wq, wk, wv, woC


def _layout_x(x, L_):
    """x [B,L,D] -> [128, 4*8192] bf16: xr[p, lp*8192 + ch*1024 + c]
    = x[lp*1024+c, ch*128+p]."""
    xT = np.ascontiguousarray(x.reshape(L_, D).T)        # [D, L]
    nlp = L_ // 1024
    xr = xT.reshape(8, 128, nlp, 1024).transpose(1, 2, 0, 3)
    return np.ascontiguousarray(xr.reshape(128, nlp * 8192)).astype(
        ml_dtypes.bfloat16)


# ---------------------------------------------------------------- device emit

def emit(nc, tc, aps, L_):
    NLP = L_ // 1024          # phase-A L tiles (4)
    NQB = L_ // 512           # query blocks (8)
    NKB = L_ // 128           # key blocks (32)
    ND = D // 128             # D chunks (8)

    xt = aps["xt"]
    partial = aps["partial"]
    ACT_EXP = mybir.ActivationFunctionType.Exp

    with tc.tile_pool(name="persist", bufs=1) as pp:
        wq_sb = pp.tile([128, D], BF16)
        wk_sb = pp.tile([128, D], BF16)
        wv_sb = pp.tile([128, D], BF16)
        wo_sb = pp.tile([128, D], BF16)
        cos_sb = pp.tile([128, L_], BF16)
        sin_sb = pp.tile([128, L_], BF16)
        tril_sb = pp.tile([128, 128], BF16)
        idb_sb = pp.tile([128, 128], BF16)
        perm_sb = pp.tile([128, 128], BF16)
        qT = pp.tile([128, L_], BF16)
        kT = pp.tile([128, L_], BF16)
        v_sb = pp.tile([128, NKB * 130], BF16)

        nc.sync.dma_start(wq_sb[:], aps["wq"][:])
        # xt for lp=0 in 8 chunk DMAs so the first matmul starts early
        xt0 = None  # created in phase A below; DMAs issued there
        nc.gpsimd.memset(v_sb[:], 1.0)   # ones columns for the sum trick

        # ---------------- Phase A: projections + RoPE + V transpose ----------
        with tc.tile_pool(name="psA", bufs=1, space="PSUM") as psA, \
             tc.tile_pool(name="sbA", bufs=1) as sbA:
            for lp in range(NLP):
                sl = bass.ts(lp, 1024)
                xt_t = sbA.tile([128, 8192], BF16, tag="xt", bufs=2)
                if lp == 0:
                    for ch in range(ND):
                        nc.sync.dma_start(xt_t[:, bass.ts(ch, 1024)],
                                          xt[:, bass.ds(lp * 8192 + ch * 1024,
                                                        1024)])
                else:
                    nc.sync.dma_start(xt_t[:], xt[:, bass.ts(lp, 8192)])
                if lp == 0:
                    # remaining persistent loads, ordered by first use
                    nc.sync.dma_start(wk_sb[:], aps["wk"][:])
                    nc.sync.dma_start(wv_sb[:], aps["wv"][:])
                    nc.sync.dma_start(cos_sb[:], aps["cos_st"][:])
                    nc.sync.dma_start(sin_sb[:], aps["sin_st"][:])
                    nc.sync.dma_start(perm_sb[:], aps["perm_b"][:])
                    nc.sync.dma_start(idb_sb[:], aps["ident_b"][:])
                    nc.sync.dma_start(tril_sb[:], aps["tril01"][:])
                    nc.sync.dma_start(wo_sb[:], aps["wo"][:])

                prs = {}
                for name, wsb in (("q", wq_sb), ("k", wk_sb), ("v", wv_sb)):
                    ps = psA.tile([128, 1024], F32, tag=name, bufs=1,
                                  name=f"pr_{name}")
                    prs[name] = ps
                    for ch in range(ND):
                        nc.tensor.matmul(ps[:], wsb[:, bass.ts(ch, 128)],
                                         xt_t[:, bass.ts(ch, 1024)],
                                         start=ch == 0, stop=ch == ND - 1)
                # RoPE: rot = raw*cos + perm(raw)*sin_signed
                for name, dst in (("q", qT), ("k", kT)):
                    raw = sbA.tile([128, 1024], BF16, tag="raw", bufs=2)
                    nc.scalar.copy(raw[:], prs[name][:])
                    aux = psA.tile([128, 1024], F32, tag="aux", bufs=1)
                    nc.tensor.matmul(aux[:], perm_sb[:], raw[:],
                                     start=True, stop=True)
                    swp = sbA.tile([128, 1024], BF16, tag="swp", bufs=2)
                    nc.vector.tensor_mul(swp[:], aux[:], sin_sb[:, sl])
                    nc.vector.tensor_mul(dst[:, sl], raw[:], cos_sb[:, sl])
                    nc.vector.tensor_add(dst[:, sl], dst[:, sl], swp[:])
                # V transpose into [key, dh] layout with ones columns:
                # v_sb[:, 130*kb + {0..63, 65..128}]
                vt = sbA.tile([128, 1024], BF16, tag="vt", bufs=2)
                nc.scalar.copy(vt[:], prs["v"][:])
                auxv = psA.tile([128, 2048], BF16, tag="aux", bufs=1)
                for j in range(8):
                    nc.tensor.transpose(auxv[:, bass.ts(j, 128)],
                                        vt[:, bass.ts(j, 128)], idb_sb[:])
                src = auxv[:, 0:1024].rearrange("p (j h c) -> p j h c",
                                                j=8, h=2)
                vdst = v_sb[:, bass.ds(130 * 8 * lp, 130 * 8)].rearrange(
                    "p (j h c) -> p j h c", j=8, c=65)[:, :, :, 0:64]
                nc.vector.tensor_copy(vdst, src)

        # ---------------- Phase B: attention + pipelined norm/o_proj ---------
        with tc.tile_pool(name="psB", bufs=1, space="PSUM") as psB, \
             tc.tile_pool(name="psNC", bufs=1, space="PSUM") as psNC, \
             tc.tile_pool(name="sbC", bufs=1) as sbC, \
             tc.tile_pool(name="sbB", bufs=1) as sbB:

            state = {}  # qb -> (O_sb, invs) for deferred norm/o_proj

            def norm_oproj(qb):
                O_t, invs = state.pop(qb)
                for qs in range(4):
                    lc = 4 * qb + qs
                    base = 130 * qs
                    for h in range(2):
                        i = 2 * qs + h
                        nc.vector.tensor_scalar_mul(
                            O_t[:, bass.ds(base + 65 * h, 64)],
                            O_t[:, bass.ds(base + 65 * h, 64)],
                            invs[:, i:i + 1])
                    trp_t = psNC.tile([128, 512], F32, tag="op", bufs=2)
                    trpb = trp_t[:].bitcast(BF16)[:, 0:128]
                    osrc = O_t[:, bass.ds(base, 130)].rearrange(
                        "p (h c) -> p h c", h=2)[:, :, 0:64]
                    nc.tensor.transpose(trpb, osrc, idb_sb[:])
                    ot_t = sbC.tile([128, 128], BF16, tag="ot", bufs=2)
                    nc.vector.tensor_copy(ot_t[:], trpb)
                    ob = sbC.tile([128, 1024], BF16, tag="ob", bufs=3)
                    for n in range(2):
                        op = psNC.tile([128, 512], F32, tag="op", bufs=2)
                        nc.tensor.matmul(op[:], ot_t[:],
                                         wo_sb[:, bass.ts(n, 512)],
                                         start=True, stop=True)
                        nc.gpsimd.tensor_copy(ob[:, bass.ts(n, 512)], op[:])
                    nc.gpsimd.dma_start(partial[bass.ts(lc, 128), :], ob[:])

            for qb in range(NQB):
                qsl0 = 512 * qb
                pvacc = psB.tile([128, 520], F32, tag="pv", bufs=1)
                nkb = 4 * qb + 4
                for kb in range(nkb):
                    r = kb - 4 * qb
                    c0 = 128 * r if r > 0 else 0
                    W = 512 - c0
                    ksl = bass.ts(kb, 128)
                    qsl = bass.ds(qsl0 + c0, W)
                    s01 = psB.tile([128, 1024], F32, tag="sc", bufs=2)
                    nc.tensor.matmul(s01[:, c0:512], kT[0:64, ksl],
                                     qT[0:64, qsl], start=True, stop=True)
                    nc.tensor.matmul(s01[:, 512 + c0:1024], kT[64:128, ksl],
                                     qT[64:128, qsl], start=True, stop=True)
                    p01 = sbB.tile([128, 1024], BF16, tag="p01", bufs=6)
                    sin_ = s01[:].rearrange("p (h c) -> p h c", h=2)[:, :, c0:512]
                    pout = p01[:].rearrange("p (h c) -> p h c", h=2)[:, :, c0:512]
                    nc.scalar.activation(pout, sin_, ACT_EXP, scale=SCALE)
                    if r >= 0:
                        for h in range(2):
                            msl = bass.ds(512 * h + c0, 128)
                            nc.vector.tensor_mul(p01[:, msl], p01[:, msl],
                                                 tril_sb[:])
                    for qs in range(max(0, r), 4):
                        for h in range(2):
                            i = 2 * qs + h
                            nc.tensor.matmul(
                                pvacc[:, bass.ds(65 * i, 65)],
                                p01[:, bass.ds(512 * h + 128 * qs, 128)],
                                v_sb[:, bass.ds(130 * kb + 65 * h, 65)],
                                start=kb == 0, stop=kb == 4 * qb + qs)
                # free pvacc quickly: reciprocal of sums + copy out
                invs = sbB.tile([128, 8], F32, tag="invs", bufs=2)
                sums = pvacc[:].rearrange("p (i c) -> p i c", c=65)[:, :, 64]
                nc.vector.reciprocal(invs[:], sums)
                O_t = sbB.tile([128, 520], BF16, tag="osb", bufs=2)
                nc.vector.tensor_copy(O_t[:], pvacc[:])
                state[qb] = (O_t, invs)
                if qb > 0:
                    norm_oproj(qb - 1)
            norm_oproj(NQB - 1)


def build(L_=L, debug=False):
    nc = bacc.Bacc("TRN2", target_bir_lowering=False, debug=debug,
                   enable_asserts=False)
    aps = {}
    NLP = L_ // 1024
    aps["xt"] = nc.dram_tensor("xt", [128, NLP * 8192], BF16,
                               kind="ExternalInput").ap()
    for w in ("wq", "wk", "wv", "wo"):
        aps[w] = nc.dram_tensor(w, [128, D], BF16, kind="ExternalInput").ap()
    aps["cos_st"] = nc.dram_tensor("cos_st", [128, L_], BF16,
                                   kind="ExternalInput").ap()
    aps["sin_st"] = nc.dram_tensor("sin_st", [128, L_], BF16,
                                   kind="ExternalInput").ap()
    aps["tril01"] = nc.dram_tensor("tril01", [128, 128], BF16,
                                   kind="ExternalInput").ap()
    aps["ident_b"] = nc.dram_tensor("ident_b", [128, 128], BF16,
                                    kind="ExternalInput").ap()
    aps["perm_b"] = nc.dram_tensor("perm_b", [128, 128], BF16,
                                   kind="ExternalInput").ap()
    aps["partial"] = nc.dram_tensor("partial", [L_, D], BF16,
                                    kind="ExternalOutput").ap()

    with tile.TileContext(nc) as tc:
        emit(nc, tc, aps, L_)
    nc.compile()
    return nc, aps


def make_in_maps(x, Wq, Wk, Wv, Wo, L_=L):
    xr = _layout_x(x, L_)
    consts = _host_consts(L_)
    in_maps = []
    for c in range(N_CORES):
        wq, wk, wv, woC = _core_weights(c, Wq, Wk, Wv, Wo)
        m = {"xt": xr, "wq": wq, "wk": wk, "wv": wv, "wo": woC}
        m.update(consts)
        in_maps.append(m)
    return in_maps


_CACHE = {}


def _run(inputs, trace=False, **kw):
    if trace:
        os.environ.pop("BASS_NEVER_TRACE", None)
    x = np.asarray(inputs["x"], np.float32)
    Wq = np.asarray(inputs["Wq"], np.float32)
    Wk = np.asarray(inputs["Wk"], np.float32)
    Wv = np.asarray(inputs["Wv"], np.float32)
    Wo = np.asarray(inputs["Wo"], np.float32)
    if "nc" not in _CACHE:
        _CACHE["nc"] = build()[0]
    nc = _CACHE["nc"]
    in_maps = make_in_maps(x, Wq, Wk, Wv, Wo)
    res = run_bass_kernel_spmd(nc, in_maps, core_ids=list(range(N_CORES)),
                               trace=trace, **kw)
    acc = np.zeros((L, D), np.float64)
    for r in res.results:
        acc += r["partial"].astype(np.float64)
    out = acc.astype(np.float32).reshape(B, L, D)
    return out, res


def kernel(**inputs):
    out, _ = _run(inputs)
    return out


# revision 10
# speedup vs baseline: 1.3621x; 1.0479x over previous
"""Trainium2 Bass kernel for nn_BloqueAttn: causal RoPE attention, 16 heads,
head-sharded (tensor-parallel) across 8 NeuronCores, o_proj row-sharded with
host-side all-reduce of the partials.

v2: bf16 datapath, query-on-partition PV (65-wide moving operand), PE
perm-matmul RoPE swap, mask-by-multiply on DVE, per-partition softmax
normalization, batched DMAs with host-side pre-layout.

Self-contained: hardcodes shapes B=1, L=4096, D=1024, H=16, DH=64, 8 cores.
"""
import os

os.environ.setdefault("BASS_NEVER_TRACE", "1")

import numpy as np
import ml_dtypes

import concourse.bass as bass
import concourse.bacc as bacc
import concourse.mybir as mybir
import concourse.tile as tile
from concourse.bass_utils import run_bass_kernel_spmd

F32 = mybir.dt.float32
BF16 = mybir.dt.bfloat16

B, L, D = 1, 4096, 1024
H, DH = 16, 64
BASE = 10000.0
N_CORES = 8
HPC = H // N_CORES          # heads per core = 2
DH2 = HPC * DH              # packed head dim = 128
SCALE = DH ** -0.5          # 0.125


# ---------------------------------------------------------------- host helpers

def _rope_tables(L_, dh):
    inv_freq = 1.0 / (BASE ** (np.arange(0, dh, 2, dtype=np.float32) / dh))
    freqs = np.outer(np.arange(L_, dtype=np.float32), inv_freq)  # [L, 32]
    return np.cos(freqs).astype(np.float32), np.sin(freqs).astype(np.float32)


def _host_consts(L_):
    cos, sin = _rope_tables(L_, DH)          # [L, 32]
    cosT, sinT = cos.T.copy(), sin.T.copy()  # [32, L]
    cos_stack = np.concatenate([cosT, cosT, cosT, cosT], 0)          # [128, L]
    sin_signed = np.concatenate([-sinT, sinT, -sinT, sinT], 0)       # [128, L]

    # 0/1 causal keep-mask within a 128x128 diagonal block:
    # key j visible to query c iff j <= c.
    j = np.arange(128)[:, None]
    c = np.arange(128)[None, :]
    tril01 = (j <= c).astype(np.float32)                             # [128,128]

    ident = np.eye(128, dtype=np.float32)
    # 32-row block swap permutation: out[i] = in[sigma(i)],
    # sigma = [32..63, 0..31, 96..127, 64..95]
    sigma = np.concatenate([np.arange(32, 64), np.arange(0, 32),
                            np.arange(96, 128), np.arange(64, 96)])
    pmat = np.zeros((128, 128), np.float32)
    pmat[sigma, np.arange(128)] = 1.0        # out = pmat.T @ in
    return {
        "cos_st": cos_stack.astype(ml_dtypes.bfloat16),
        "sin_st": sin_signed.astype(ml_dtypes.bfloat16),
        "tril01": tril01.astype(ml_dtypes.bfloat16),
        "ident_b": ident.astype(ml_dtypes.bfloat16),
        "perm_b": pmat.astype(ml_dtypes.bfloat16),
    }


def _chunk_major(wT):
    """[D, 128] -> [128, D] with 128-row chunks laid side by side."""
    ndc = wT.shape[0] // 128
    return np.ascontiguousarray(
        wT.reshape(ndc, 128, 128).transpose(1, 0, 2).reshape(128, ndc * 128))


def _core_weights(core, Wq, Wk, Wv, Wo):
    """Per-core weight slices, bf16, chunk-major; RoPE even/odd permutation
    applied to Wq/Wk rows."""
    perm = np.concatenate([np.arange(0, DH, 2), np.arange(1, DH, 2)])  # [64]
    rows_p, rows = [], []
    for hh in (HPC * core, HPC * core + 1):
        rows_p.append(DH * hh + perm)
        rows.append(DH * hh + np.arange(DH))
    rows_p = np.concatenate(rows_p)
    rows = np.concatenate(rows)
    wq = _chunk_major(Wq[rows_p, :].T).astype(ml_dtypes.bfloat16)  # [128, 1024]
    wk = _chunk_major(Wk[rows_p, :].T).astype(ml_dtypes.bfloat16)
    wv = _chunk_major(Wv[rows, :].T).astype(ml_dtypes.bfloat16)
    woC = np.ascontiguousarray(
        Wo[:, DH2 * core: DH2 * (core + 1)].T).astype(ml_dtypes.bfloat16)
    return wq, wk, wv, woC


def _layout_x(x, L_):
    """x [B,L,D] -> [128, 4*8192] bf16: xr[p, lp*8192 + ch*1024 + c]
    = x[lp*1024+c, ch*128+p]."""
    xT = np.ascontiguousarray(x.reshape(L_, D).T)        # [D, L]
    nlp = L_ // 1024
    xr = xT.reshape(8, 128, nlp, 1024).transpose(1, 2, 0, 3)
    return np.ascontiguousarray(xr.reshape(128, nlp * 8192)).astype(
        ml_dtypes.bfloat16)


# ---------------------------------------------------------------- device emit

def emit(nc, tc, aps, L_):
    NLP = L_ // 1024          # phase-A L tiles (4)
    NQB = L_ // 512           # query blocks (8)
    NKB = L_ // 128           # key blocks (32)
    ND = D // 128             # D chunks (8)

    xt = aps["xt"]
    partial = aps["partial"]
    ACT_EXP = mybir.ActivationFunctionType.Exp

    with tc.tile_pool(name="persist", bufs=1) as pp:
        wq_sb = pp.tile([128, D], BF16)
        wk_sb = pp.tile([128, D], BF16)
        wv_sb = pp.tile([128, D], BF16)
        wo_sb = pp.tile([128, D], BF16)
        cos_sb = pp.tile([128, L_], BF16)
        sin_sb = pp.tile([128, L_], BF16)
        tril_sb = pp.tile([128, 128], BF16)
        idb_sb = pp.tile([128, 128], BF16)
        perm_sb = pp.tile([128, 128], BF16)
        qT = pp.tile([128, L_], BF16)
        kT = pp.tile([128, L_], BF16)
        v_sb = pp.tile([128, NKB * 130], BF16)
        zer = pp.tile([128, 512], BF16)
        nc.gpsimd.memset(zer[:], 0.0)

        nc.sync.dma_start(wq_sb[:], aps["wq"][:])
        # xt for lp=0 in 8 chunk DMAs so the first matmul starts early
        xt0 = None  # created in phase A below; DMAs issued there
        nc.gpsimd.memset(v_sb[:], 1.0)   # ones columns for the sum trick

        # ---------------- Phase A: projections + RoPE + V transpose ----------
        with tc.tile_pool(name="psA", bufs=1, space="PSUM") as psA, \
             tc.tile_pool(name="sbA", bufs=1) as sbA:
            for lp in range(NLP):
                sl = bass.ts(lp, 1024)
                xt_t = sbA.tile([128, 8192], BF16, tag="xt", bufs=2)
                if lp == 0:
                    for ch in range(ND):
                        nc.sync.dma_start(xt_t[:, bass.ts(ch, 1024)],
                                          xt[:, bass.ds(lp * 8192 + ch * 1024,
                                                        1024)])
                else:
                    nc.sync.dma_start(xt_t[:], xt[:, bass.ts(lp, 8192)])
                if lp == 0:
                    # remaining persistent loads, ordered by first use
                    nc.sync.dma_start(wk_sb[:], aps["wk"][:])
                    nc.sync.dma_start(wv_sb[:], aps["wv"][:])
                    nc.sync.dma_start(cos_sb[:], aps["cos_st"][:])
                    nc.sync.dma_start(sin_sb[:], aps["sin_st"][:])
                    nc.sync.dma_start(perm_sb[:], aps["perm_b"][:])
                    nc.sync.dma_start(idb_sb[:], aps["ident_b"][:])
                    nc.sync.dma_start(tril_sb[:], aps["tril01"][:])
                    nc.sync.dma_start(wo_sb[:], aps["wo"][:])

                prs = {}
                for name, wsb in (("q", wq_sb), ("k", wk_sb), ("v", wv_sb)):
                    ps = psA.tile([128, 1024], F32, tag=name, bufs=1,
                                  name=f"pr_{name}")
                    prs[name] = ps
                    for ch in range(ND):
                        for hf in range(2):
                            nc.tensor.matmul(
                                ps[:, bass.ts(hf, 512)],
                                wsb[:, bass.ts(ch, 128)],
                                xt_t[:, bass.ds(ch * 1024 + hf * 512, 512)],
                                start=ch == 0, stop=ch == ND - 1)
                # RoPE: rot = raw*cos + perm(raw)*sin_signed
                for name, dst in (("q", qT), ("k", kT)):
                    raw = sbA.tile([128, 1024], BF16, tag="raw", bufs=2)
                    nc.scalar.copy(raw[:], prs[name][:])
                    aux = psA.tile([128, 1024], F32, tag="aux", bufs=1)
                    for hf in range(2):
                        nc.tensor.matmul(aux[:, bass.ts(hf, 512)], perm_sb[:],
                                         raw[:, bass.ts(hf, 512)],
                                         start=True, stop=True)
                    swp = sbA.tile([128, 1024], BF16, tag="swp", bufs=2)
                    nc.vector.tensor_mul(swp[:], aux[:], sin_sb[:, sl])
                    nc.vector.tensor_mul(dst[:, sl], raw[:], cos_sb[:, sl])
                    nc.vector.tensor_add(dst[:, sl], dst[:, sl], swp[:])
                # V transpose into [key, dh] layout with ones columns:
                # v_sb[:, 130*kb + {0..63, 65..128}]
                vt = sbA.tile([128, 1024], BF16, tag="vt", bufs=2)
                nc.scalar.copy(vt[:], prs["v"][:])
                auxv = psA.tile([128, 2048], BF16, tag="aux", bufs=1)
                for j in range(8):
                    nc.tensor.transpose(auxv[:, bass.ts(j, 128)],
                                        vt[:, bass.ts(j, 128)], idb_sb[:])
                src = auxv[:, 0:1024].rearrange("p (j h c) -> p j h c",
                                                j=8, h=2)
                vdst = v_sb[:, bass.ds(130 * 8 * lp, 130 * 8)].rearrange(
                    "p (j h c) -> p j h c", j=8, c=65)[:, :, :, 0:64]
                nc.vector.tensor_copy(vdst, src)

        # ---------------- Phase B: attention + pipelined norm/o_proj ---------
        with tc.tile_pool(name="psB", bufs=1, space="PSUM") as psB, \
             tc.tile_pool(name="psNC", bufs=1, space="PSUM") as psNC, \
             tc.tile_pool(name="sbC", bufs=1) as sbC, \
             tc.tile_pool(name="sbB", bufs=1) as sbB:

            state = {}  # qb -> (O_sb, invs) for deferred norm/o_proj

            def norm_oproj(qb):
                O_t, invs = state.pop(qb)
                for qs in range(4):
                    lc = 4 * qb + qs
                    for h in range(2):
                        i = 2 * qs + h
                        nc.vector.tensor_scalar_mul(
                            O_t[:, bass.ds(64 * i, 64)],
                            O_t[:, bass.ds(64 * i, 64)],
                            invs[:, i:i + 1])
                    trp_t = psNC.tile([128, 512], F32, tag="op", bufs=2)
                    trpb = trp_t[:].bitcast(BF16)[:, 0:128]
                    nc.tensor.transpose(trpb, O_t[:, bass.ts(qs, 128)],
                                        idb_sb[:])
                    ot_t = sbC.tile([128, 128], BF16, tag="ot", bufs=2)
                    nc.vector.tensor_copy(ot_t[:], trpb)
                    ob = sbC.tile([128, 1024], BF16, tag="ob", bufs=3)
                    for n in range(2):
                        op = psNC.tile([128, 512], F32, tag="op", bufs=2)
                        nc.tensor.matmul(op[:], ot_t[:],
                                         wo_sb[:, bass.ts(n, 512)],
                                         start=True, stop=True)
                        nc.vector.tensor_copy(ob[:, bass.ts(n, 512)], op[:])
                    nc.gpsimd.dma_start(partial[bass.ts(lc, 128), :], ob[:])

            # PSUM accumulate-group state is per bank: region 7 would cross
            # the 2048B bank boundary at col 455, so it lives at col 512.
            PVC = [65 * i for i in range(7)] + [512]
            for qb in range(NQB):
                qsl0 = 512 * qb
                pvacc = psB.tile([128, 577], F32, tag="pv", bufs=1)
                nc.tensor.matmul(pvacc[:, 0:512], zer[:, 0:128], zer[:],
                                 start=True, stop=False, skip_group_check=True)
                nc.tensor.matmul(pvacc[:, 512:577], zer[:, 0:128],
                                 zer[:, 0:65], start=True, stop=False,
                                 skip_group_check=True)
                nkb = 4 * qb + 4
                for kb in range(nkb):
                    r = kb - 4 * qb
                    c0 = 128 * r if r > 0 else 0
                    W = 512 - c0
                    ksl = bass.ts(kb, 128)
                    qsl = bass.ds(qsl0 + c0, W)
                    s01 = psB.tile([128, 1024], F32, tag="sc", bufs=2)
                    nc.tensor.matmul(s01[:, c0:512], kT[0:64, ksl],
                                     qT[0:64, qsl], start=True, stop=True)
                    nc.tensor.matmul(s01[:, 512 + c0:1024], kT[64:128, ksl],
                                     qT[64:128, qsl], start=True, stop=True)
                    p01 = sbB.tile([128, 1024], BF16, tag="p01", bufs=6)
                    sin_ = s01[:].rearrange("p (h c) -> p h c", h=2)[:, :, c0:512]
                    pout = p01[:].rearrange("p (h c) -> p h c", h=2)[:, :, c0:512]
                    nc.scalar.activation(pout, sin_, ACT_EXP, scale=SCALE)
                    if r >= 0:
                        for h in range(2):
                            msl = bass.ds(512 * h + c0, 128)
                            nc.vector.tensor_mul(p01[:, msl], p01[:, msl],
                                                 tril_sb[:])
                    for qs in range(max(0, r), 4):
                        for h in range(2):
                            i = 2 * qs + h
                            nc.tensor.matmul(
                                pvacc[:, bass.ds(PVC[i], 65)],
                                p01[:, bass.ds(512 * h + 128 * qs, 128)],
                                v_sb[:, bass.ds(130 * kb + 65 * h, 65)],
                                start=False, stop=kb == 4 * qb + qs,
                                skip_group_check=True)
                # free pvacc quickly: reciprocal of sums + copy out
                invs = sbB.tile([128, 8], F32, tag="invs", bufs=2)
                sums7 = pvacc[:, 0:455].rearrange(
                    "p (i c) -> p i c", c=65)[:, :, 64]
                nc.vector.reciprocal(invs[:, 0:7], sums7)
                nc.vector.reciprocal(invs[:, 7:8], pvacc[:, 576:577])
                O_t = sbB.tile([128, 512], BF16, tag="osb", bufs=2)
                psrc7 = pvacc[:, 0:455].rearrange(
                    "p (i c) -> p i c", c=65)[:, :, 0:64]
                nc.vector.tensor_copy(
                    O_t[:, 0:448].rearrange("p (i c) -> p i c", c=64), psrc7)
                nc.vector.tensor_copy(O_t[:, 448:512], pvacc[:, 512:576])
                state[qb] = (O_t, invs)
                if qb > 0:
                    norm_oproj(qb - 1)
            norm_oproj(NQB - 1)


def build(L_=L, debug=False):
    nc = bacc.Bacc("TRN2", target_bir_lowering=False, debug=debug,
                   enable_asserts=False)
    aps = {}
    NLP = L_ // 1024
    aps["xt"] = nc.dram_tensor("xt", [128, NLP * 8192], BF16,
                               kind="ExternalInput").ap()
    for w in ("wq", "wk", "wv", "wo"):
        aps[w] = nc.dram_tensor(w, [128, D], BF16, kind="ExternalInput").ap()
    aps["cos_st"] = nc.dram_tensor("cos_st", [128, L_], BF16,
                                   kind="ExternalInput").ap()
    aps["sin_st"] = nc.dram_tensor("sin_st", [128, L_], BF16,
                                   kind="ExternalInput").ap()
    aps["tril01"] = nc.dram_tensor("tril01", [128, 128], BF16,
                                   kind="ExternalInput").ap()
    aps["ident_b"] = nc.dram_tensor("ident_b", [128, 128], BF16,
                                    kind="ExternalInput").ap()
    aps["perm_b"] = nc.dram_tensor("perm_b", [128, 128], BF16,
                                   kind="ExternalInput").ap()
    aps["partial"] = nc.dram_tensor("partial", [L_, D], BF16,
                                    kind="ExternalOutput").ap()

    with tile.TileContext(nc) as tc:
        emit(nc, tc, aps, L_)
    nc.compile()
    return nc, aps


def make_in_maps(x, Wq, Wk, Wv, Wo, L_=L):
    xr = _layout_x(x, L_)
    consts = _host_consts(L_)
    in_maps = []
    for c in range(N_CORES):
        wq, wk, wv, woC = _core_weights(c, Wq, Wk, Wv, Wo)
        m = {"xt": xr, "wq": wq, "wk": wk, "wv": wv, "wo": woC}
        m.update(consts)
        in_maps.append(m)
    return in_maps


_CACHE = {}


def _run(inputs, trace=False, **kw):
    if trace:
        os.environ.pop("BASS_NEVER_TRACE", None)
    x = np.asarray(inputs["x"], np.float32)
    Wq = np.asarray(inputs["Wq"], np.float32)
    Wk = np.asarray(inputs["Wk"], np.float32)
    Wv = np.asarray(inputs["Wv"], np.float32)
    Wo = np.asarray(inputs["Wo"], np.float32)
    if "nc" not in _CACHE:
        _CACHE["nc"] = build()[0]
    nc = _CACHE["nc"]
    in_maps = make_in_maps(x, Wq, Wk, Wv, Wo)
    res = run_bass_kernel_spmd(nc, in_maps, core_ids=list(range(N_CORES)),
                               trace=trace, **kw)
    acc = np.zeros((L, D), np.float64)
    for r in res.results:
        acc += r["partial"].astype(np.float64)
    out = acc.astype(np.float32).reshape(B, L, D)
    return out, res


def kernel(**inputs):
    out, _ = _run(inputs)
    return out


# revision 13
# speedup vs baseline: 1.4290x; 1.0491x over previous
"""Trainium2 Bass kernel for nn_BloqueAttn: causal RoPE attention, 16 heads,
head-sharded (tensor-parallel) across 8 NeuronCores, o_proj row-sharded with
host-side all-reduce of the partials.

v2: bf16 datapath, query-on-partition PV (65-wide moving operand), PE
perm-matmul RoPE swap, mask-by-multiply on DVE, per-partition softmax
normalization, batched DMAs with host-side pre-layout.

Self-contained: hardcodes shapes B=1, L=4096, D=1024, H=16, DH=64, 8 cores.
"""
import os

os.environ.setdefault("BASS_NEVER_TRACE", "1")

import numpy as np
import ml_dtypes

import concourse.bass as bass
import concourse.bacc as bacc
import concourse.mybir as mybir
import concourse.tile as tile
from concourse.bass_utils import run_bass_kernel_spmd

F32 = mybir.dt.float32
BF16 = mybir.dt.bfloat16

B, L, D = 1, 4096, 1024
H, DH = 16, 64
BASE = 10000.0
N_CORES = 8
HPC = H // N_CORES          # heads per core = 2
DH2 = HPC * DH              # packed head dim = 128
SCALE = DH ** -0.5          # 0.125


# ---------------------------------------------------------------- host helpers

def _rope_tables(L_, dh):
    inv_freq = 1.0 / (BASE ** (np.arange(0, dh, 2, dtype=np.float32) / dh))
    freqs = np.outer(np.arange(L_, dtype=np.float32), inv_freq)  # [L, 32]
    return np.cos(freqs).astype(np.float32), np.sin(freqs).astype(np.float32)


def _host_consts(L_):
    cos, sin = _rope_tables(L_, DH)          # [L, 32]
    cosT, sinT = cos.T.copy(), sin.T.copy()  # [32, L]
    cos_stack = np.concatenate([cosT, cosT, cosT, cosT], 0)          # [128, L]
    sin_signed = np.concatenate([-sinT, sinT, -sinT, sinT], 0)       # [128, L]

    # 0/1 causal keep-mask within a 128x128 diagonal block:
    # key j visible to query c iff j <= c.
    j = np.arange(128)[:, None]
    c = np.arange(128)[None, :]
    tril01 = (j <= c).astype(np.float32)                             # [128,128]

    ident = np.eye(128, dtype=np.float32)
    # 32-row block swap permutation: out[i] = in[sigma(i)],
    # sigma = [32..63, 0..31, 96..127, 64..95]
    sigma = np.concatenate([np.arange(32, 64), np.arange(0, 32),
                            np.arange(96, 128), np.arange(64, 96)])
    pmat = np.zeros((128, 128), np.float32)
    pmat[sigma, np.arange(128)] = 1.0        # out = pmat.T @ in
    return {
        "cos_st": cos_stack.astype(ml_dtypes.bfloat16),
        "sin_st": sin_signed.astype(ml_dtypes.bfloat16),
        "tril01": tril01.astype(ml_dtypes.bfloat16),
        "ident_b": ident.astype(ml_dtypes.bfloat16),
        "perm_b": pmat.astype(ml_dtypes.bfloat16),
    }


def _chunk_major(wT):
    """[D, 128] -> [128, D] with 128-row chunks laid side by side."""
    ndc = wT.shape[0] // 128
    return np.ascontiguousarray(
        wT.reshape(ndc, 128, 128).transpose(1, 0, 2).reshape(128, ndc * 128))


def _core_weights(core, Wq, Wk, Wv, Wo):
    """Per-core weight slices, bf16, chunk-major; RoPE even/odd permutation
    applied to Wq/Wk rows."""
    perm = np.concatenate([np.arange(0, DH, 2), np.arange(1, DH, 2)])  # [64]
    rows_p, rows = [], []
    for hh in (HPC * core, HPC * core + 1):
        rows_p.append(DH * hh + perm)
        rows.append(DH * hh + np.arange(DH))
    rows_p = np.concatenate(rows_p)
    rows = np.concatenate(rows)
    wq = _chunk_major(Wq[rows_p, :].T).astype(ml_dtypes.bfloat16)  # [128, 1024]
    wk = _chunk_major(Wk[rows_p, :].T).astype(ml_dtypes.bfloat16)
    wv = _chunk_major(Wv[rows, :].T).astype(ml_dtypes.bfloat16)
    woC = np.ascontiguousarray(
        Wo[:, DH2 * core: DH2 * (core + 1)].T).astype(ml_dtypes.bfloat16)
    return wq, wk, wv, woC


def _layout_x(x, L_):
    """x [B,L,D] -> [128, 8*4096] bf16, 512-col subtile-major:
    xr[p, s*4096 + ch*512 + c] = x[s*512+c, ch*128+p]."""
    xT = np.ascontiguousarray(x.reshape(L_, D).T)        # [D, L]
    ns = L_ // 512
    xr = xT.reshape(8, 128, ns, 512).transpose(1, 2, 0, 3)
    return np.ascontiguousarray(xr.reshape(128, ns * 4096)).astype(
        ml_dtypes.bfloat16)


# ---------------------------------------------------------------- device emit

def emit(nc, tc, aps, L_):
    NSB = L_ // 512           # 512-col subtiles (8) == query blocks
    NQB = L_ // 512
    NKB = L_ // 128           # key blocks (32)
    ND = D // 128             # D chunks (8)

    xt = aps["xt"]
    partial = aps["partial"]
    ACT_EXP = mybir.ActivationFunctionType.Exp

    with tc.tile_pool(name="persist", bufs=1) as pp, \
         tc.tile_pool(name="psB", bufs=1, space="PSUM") as psB, \
         tc.tile_pool(name="psS", bufs=1, space="PSUM") as psS, \
         tc.tile_pool(name="sbC", bufs=1) as sbC, \
         tc.tile_pool(name="sbB", bufs=1) as sbB, \
         tc.tile_pool(name="sbA", bufs=1) as sbA:
        wq_sb = pp.tile([128, D], BF16)
        wk_sb = pp.tile([128, D], BF16)
        wv_sb = pp.tile([128, D], BF16)
        wo_sb = pp.tile([128, D], BF16)
        cos_sb = pp.tile([128, L_], BF16)
        sin_sb = pp.tile([128, L_], BF16)
        tril_sb = pp.tile([128, 128], BF16)
        idb_sb = pp.tile([128, 128], BF16)
        perm_sb = pp.tile([128, 128], BF16)
        qT = pp.tile([128, L_], BF16)
        kT = pp.tile([128, L_], BF16)
        v_sb = pp.tile([128, NKB * 130], BF16)
        zer = pp.tile([128, 512], BF16)
        nc.gpsimd.memset(zer[:], 0.0)
        nc.sync.dma_start(wq_sb[:], aps["wq"][:])
        nc.gpsimd.memset(v_sb[:], 1.0)   # ones columns for the sum trick

        def phase_a(s):
            """Projections + RoPE + V transpose for L-subtile s (512 cols)."""
            sl = bass.ds(512 * s, 512)
            xt_t = sbA.tile([128, 4096], BF16, tag="xt", bufs=2)
            if s == 0:
                for ch in range(ND):
                    nc.sync.dma_start(xt_t[:, bass.ts(ch, 512)],
                                      xt[:, bass.ds(ch * 512, 512)])
                nc.sync.dma_start(wk_sb[:], aps["wk"][:])
                nc.sync.dma_start(wv_sb[:], aps["wv"][:])
                nc.sync.dma_start(cos_sb[:], aps["cos_st"][:])
                nc.sync.dma_start(sin_sb[:], aps["sin_st"][:])
                nc.sync.dma_start(perm_sb[:], aps["perm_b"][:])
                nc.sync.dma_start(idb_sb[:], aps["ident_b"][:])
                nc.sync.dma_start(tril_sb[:], aps["tril01"][:])
                nc.sync.dma_start(wo_sb[:], aps["wo"][:])
            else:
                nc.sync.dma_start(xt_t[:], xt[:, bass.ts(s, 4096)])
            raws = {}
            for name, wsb in (("q", wq_sb), ("k", wk_sb), ("v", wv_sb)):
                ps = psS.tile([128, 512], F32, tag="scr", bufs=2)
                for ch in range(ND):
                    nc.tensor.matmul(ps[:], wsb[:, bass.ts(ch, 128)],
                                     xt_t[:, bass.ts(ch, 512)],
                                     start=ch == 0, stop=ch == ND - 1)
                raw = sbA.tile([128, 512], BF16, tag=f"raw{name}", bufs=2)
                nc.vector.tensor_copy(raw[:], ps[:])
                raws[name] = raw
            # RoPE: rot = raw*cos + perm(raw)*sin_signed
            for name, dst in (("q", qT), ("k", kT)):
                raw = raws[name]
                aux = psS.tile([128, 512], F32, tag="scr", bufs=2)
                nc.tensor.matmul(aux[:], perm_sb[:], raw[:],
                                 start=True, stop=True)
                swp = sbA.tile([128, 512], BF16, tag="swp", bufs=2)
                nc.vector.tensor_mul(swp[:], aux[:], sin_sb[:, sl])
                nc.vector.tensor_mul(dst[:, sl], raw[:], cos_sb[:, sl])
                nc.vector.tensor_add(dst[:, sl], dst[:, sl], swp[:])
            # V transpose into [key, dh] layout with ones columns:
            # v_sb[:, 130*kb + {0..63, 65..128}], kb = 4*s + j
            auxv_t = psS.tile([128, 512], F32, tag="scr", bufs=2)
            auxv = auxv_t[:].bitcast(BF16)[:, 0:512]
            vt = raws["v"]
            for j in range(4):
                nc.tensor.transpose(auxv[:, bass.ts(j, 128)],
                                    vt[:, bass.ts(j, 128)], idb_sb[:])
            src = auxv.rearrange("p (j h c) -> p j h c", j=4, h=2)
            vdst = v_sb[:, bass.ds(130 * 4 * s, 130 * 4)].rearrange(
                "p (j h c) -> p j h c", j=4, c=65)[:, :, :, 0:64]
            nc.vector.tensor_copy(vdst, src)

        state = {}  # qb -> (O_sb, invs) for deferred norm/o_proj

        def norm_oproj(qb, use_act=False):
            O_t, invs = state.pop(qb)
            cpy = nc.scalar.copy if use_act else nc.vector.tensor_copy
            for qs in range(4):
                lc = 4 * qb + qs
                for h in range(2):
                    i = 2 * qs + h
                    nc.vector.tensor_scalar_mul(
                        O_t[:, bass.ds(64 * i, 64)],
                        O_t[:, bass.ds(64 * i, 64)],
                        invs[:, i:i + 1])
                trp_t = psS.tile([128, 512], F32, tag="scr", bufs=2)
                trpb = trp_t[:].bitcast(BF16)[:, 0:128]
                nc.tensor.transpose(trpb, O_t[:, bass.ts(qs, 128)],
                                    idb_sb[:])
                ot_t = sbC.tile([128, 128], BF16, tag="ot", bufs=2)
                nc.vector.tensor_copy(ot_t[:], trpb)
                ob = sbC.tile([128, 1024], BF16, tag="ob", bufs=3)
                for n in range(2):
                    op = psS.tile([128, 512], F32, tag="scr", bufs=2)
                    nc.tensor.matmul(op[:], ot_t[:],
                                     wo_sb[:, bass.ts(n, 512)],
                                     start=True, stop=True)
                    cpy(ob[:, bass.ts(n, 512)], op[:])
                nc.gpsimd.dma_start(partial[bass.ts(lc, 128), :], ob[:])

        # PSUM accumulate-group state is per bank: region 7 would cross
        # the 2048B bank boundary at col 455, so it lives at col 512.
        PVC = [65 * i for i in range(7)] + [512]

        def attention(qb):
            qsl0 = 512 * qb
            pvacc = psB.tile([128, 577], F32, tag="pv", bufs=1)
            nc.tensor.matmul(pvacc[:, 0:512], zer[:, 0:128], zer[:],
                             start=True, stop=False, skip_group_check=True)
            nc.tensor.matmul(pvacc[:, 512:577], zer[:, 0:128],
                             zer[:, 0:65], start=True, stop=False,
                             skip_group_check=True)
            nkb = 4 * qb + 4
            for kb in range(nkb):
                r = kb - 4 * qb
                c0 = 128 * r if r > 0 else 0
                W = 512 - c0
                ksl = bass.ts(kb, 128)
                qsl = bass.ds(qsl0 + c0, W)
                s01 = psB.tile([128, 1024], F32, tag="sc", bufs=2)
                nc.tensor.matmul(s01[:, c0:512], kT[0:64, ksl],
                                 qT[0:64, qsl], start=True, stop=True)
                nc.tensor.matmul(s01[:, 512 + c0:1024], kT[64:128, ksl],
                                 qT[64:128, qsl], start=True, stop=True)
                p01 = sbB.tile([128, 1024], BF16, tag="p01", bufs=6)
                sin_ = s01[:].rearrange("p (h c) -> p h c", h=2)[:, :, c0:512]
                pout = p01[:].rearrange("p (h c) -> p h c", h=2)[:, :, c0:512]
                nc.scalar.activation(pout, sin_, ACT_EXP, scale=SCALE)
                if r >= 0:
                    for h in range(2):
                        msl = bass.ds(512 * h + c0, 128)
                        nc.vector.tensor_mul(p01[:, msl], p01[:, msl],
                                             tril_sb[:])
                for qs in range(max(0, r), 4):
                    for h in range(2):
                        i = 2 * qs + h
                        nc.tensor.matmul(
                            pvacc[:, bass.ds(PVC[i], 65)],
                            p01[:, bass.ds(512 * h + 128 * qs, 128)],
                            v_sb[:, bass.ds(130 * kb + 65 * h, 65)],
                            start=False, stop=kb == 4 * qb + qs,
                            skip_group_check=True)
            # free pvacc quickly: reciprocal of sums + copy out
            invs = sbB.tile([128, 8], F32, tag="invs", bufs=2)
            sums7 = pvacc[:, 0:455].rearrange(
                "p (i c) -> p i c", c=65)[:, :, 64]
            nc.vector.reciprocal(invs[:, 0:7], sums7)
            nc.vector.reciprocal(invs[:, 7:8], pvacc[:, 576:577])
            O_t = sbB.tile([128, 512], BF16, tag="osb", bufs=2)
            psrc7 = pvacc[:, 0:455].rearrange(
                "p (i c) -> p i c", c=65)[:, :, 0:64]
            nc.vector.tensor_copy(
                O_t[:, 0:448].rearrange("p (i c) -> p i c", c=64), psrc7)
            nc.vector.tensor_copy(O_t[:, 448:512], pvacc[:, 512:576])
            state[qb] = (O_t, invs)

        # Interleave: subtile s unlocks attention row qb=s-1 (keys for row
        # qb fully available once subtiles 0..qb are projected).
        for s in range(NSB):
            phase_a(s)
            if s >= 1:
                attention(s - 1)
            if s >= 2:
                norm_oproj(s - 2)
        attention(NQB - 1)
        norm_oproj(NQB - 2, use_act=True)
        norm_oproj(NQB - 1, use_act=True)


def build(L_=L, debug=False):
    nc = bacc.Bacc("TRN2", target_bir_lowering=False, debug=debug,
                   enable_asserts=False)
    aps = {}
    NSB = L_ // 512
    aps["xt"] = nc.dram_tensor("xt", [128, NSB * 4096], BF16,
                               kind="ExternalInput").ap()
    for w in ("wq", "wk", "wv", "wo"):
        aps[w] = nc.dram_tensor(w, [128, D], BF16, kind="ExternalInput").ap()
    aps["cos_st"] = nc.dram_tensor("cos_st", [128, L_], BF16,
                                   kind="ExternalInput").ap()
    aps["sin_st"] = nc.dram_tensor("sin_st", [128, L_], BF16,
                                   kind="ExternalInput").ap()
    aps["tril01"] = nc.dram_tensor("tril01", [128, 128], BF16,
                                   kind="ExternalInput").ap()
    aps["ident_b"] = nc.dram_tensor("ident_b", [128, 128], BF16,
                                    kind="ExternalInput").ap()
    aps["perm_b"] = nc.dram_tensor("perm_b", [128, 128], BF16,
                                   kind="ExternalInput").ap()
    aps["partial"] = nc.dram_tensor("partial", [L_, D], BF16,
                                    kind="ExternalOutput").ap()

    with tile.TileContext(nc) as tc:
        emit(nc, tc, aps, L_)
    nc.compile()
    return nc, aps


def make_in_maps(x, Wq, Wk, Wv, Wo, L_=L):
    xr = _layout_x(x, L_)
    consts = _host_consts(L_)
    in_maps = []
    for c in range(N_CORES):
        wq, wk, wv, woC = _core_weights(c, Wq, Wk, Wv, Wo)
        m = {"xt": xr, "wq": wq, "wk": wk, "wv": wv, "wo": woC}
        m.update(consts)
        in_maps.append(m)
    return in_maps


_CACHE = {}


def _run(inputs, trace=False, **kw):
    if trace:
        os.environ.pop("BASS_NEVER_TRACE", None)
    x = np.asarray(inputs["x"], np.float32)
    Wq = np.asarray(inputs["Wq"], np.float32)
    Wk = np.asarray(inputs["Wk"], np.float32)
    Wv = np.asarray(inputs["Wv"], np.float32)
    Wo = np.asarray(inputs["Wo"], np.float32)
    if "nc" not in _CACHE:
        _CACHE["nc"] = build()[0]
    nc = _CACHE["nc"]
    in_maps = make_in_maps(x, Wq, Wk, Wv, Wo)
    res = run_bass_kernel_spmd(nc, in_maps, core_ids=list(range(N_CORES)),
                               trace=trace, **kw)
    acc = np.zeros((L, D), np.float64)
    for r in res.results:
        acc += r["partial"].astype(np.float64)
    out = acc.astype(np.float32).reshape(B, L, D)
    return out, res


def kernel(**inputs):
    out, _ = _run(inputs)
    return out
